# revision 5
# baseline (speedup 1.0000x reference)
"""NodeUnpool kernel for 8 Trainium2 NeuronCores (Bass/Tile, SPMD).

Computation (see nn.Module reference):
    old = h_full[old_idxs]                      # [M, 256] gather
    merged = old @ W1.T + b1 + h_sub @ W2.T + b2
    out = h_full with rows old_idxs replaced by merged

Strategy:
  * old_idxs is arange(M) in this problem (fill="arange"), so the gather and
    scatter are contiguous row slices. A general host-side gather/scatter
    fallback handles any other index pattern.
  * The device work is exactly the merged-row GEMM: X=[old | h_sub] [M,512]
    @ Wc.T + (b1+b2), sharded row-wise across 8 cores (M/8 = 31250 rows each).
  * Activations travel as float16 (the 2e-2 rel-err budget dwarfs fp16's
    ~1e-3 contribution), halving HBM traffic vs fp32 — this kernel is
    DMA-bound, so bytes are time. Outputs also return as fp16.
  * Layout is feature-major and k-tile-packed: xT[p, kt, r] = X[r, kt*128+p]
    so chunk DMAs are few and large (1-2 MiB; big dma_starts split across all
    16 SDMA engines and run near peak HBM BW). Inputs issue on the SP HWDGE
    ring, outputs on the Activation ring — HWDGE is FIFO per issuing engine,
    so separate rings let in/out transfers overlap; per-kt input DMAs let the
    first matmul start after 1/4 of the chunk has landed.
  * PE runs fp16 at 1 cycle/row (full rate); PSUM accumulates fp32; the DVE
    adds the bias and converts fp32->fp16 during PSUM eviction.
  * Pass-through rows (h_full[M:]) never touch the device; they are copied on
    the host during output assembly.
"""

import sys
from concurrent.futures import ThreadPoolExecutor

import numpy as np

N, M, DIM = 1_000_000, 250_000, 256
N_CORES = 8
R = M // N_CORES        # 31250 merged rows per core
CHUNK = 4096            # rows of X processed per inner step
KT = (2 * DIM) // 128   # 4 contraction tiles
JT = DIM // 128         # 2 output-feature blocks
NBLK = 512              # PSUM bank free-dim (fp32)

_NC_CACHE = {}
_POOL = ThreadPoolExecutor(max_workers=N_CORES)


def _ensure_concourse():
    try:
        import concourse.bass  # noqa: F401
    except ImportError:  # pragma: no cover
        sys.path.insert(0, "/opt/trn_rl_repo")
        import concourse.bass  # noqa: F401


def _build_nc(repeat=1, hw_loop=1):
    """Build + bacc-compile the per-core Bass program (identical on all cores).

    repeat>1 unrolls the whole compute loop k times inside one NEFF (same
    inputs/outputs each pass); hw_loop>1 additionally wraps those k unrolled
    passes in a tc.For_i hardware loop — total passes = repeat*hw_loop with
    O(repeat) code size. Used by dev_hwtime for slope timing.
    """
    _ensure_concourse()
    import concourse.bacc as bacc
    import concourse.tile as tile
    from concourse import mybir

    f16 = mybir.dt.float16
    f32 = mybir.dt.float32

    nc = bacc.Bacc("TRN2", target_bir_lowering=False, debug=False)
    xT = nc.dram_tensor("xT", [128, KT, R], f16, kind="ExternalInput")
    wT = nc.dram_tensor("wT", [128, KT, DIM], f16, kind="ExternalInput")
    bias = nc.dram_tensor("bias", [128, JT], f32, kind="ExternalInput")
    outT = nc.dram_tensor("outT", [128, JT, R], f16, kind="ExternalOutput")

    with tile.TileContext(nc) as tc:
        with (
            tc.tile_pool(name="wpool", bufs=1) as wpool,
            tc.tile_pool(name="io", bufs=2) as io,
            tc.tile_pool(name="pp", bufs=8, space="PSUM") as pp,
        ):
            w_sb = wpool.tile([128, KT, DIM], f16)
            nc.sync.dma_start(out=w_sb[:], in_=wT[:])
            b_sb = wpool.tile([128, JT], f32)
            nc.sync.dma_start(out=b_sb[:], in_=bias[:])

            def body():
                for _rep in range(repeat):
                    col = 0
                    while col < R:
                        ch = min(CHUNK, R - col)
                        xt = io.tile([128, KT, CHUNK], f16, tag="x", name="x")
                        for kt in range(KT):
                            nc.sync.dma_start(
                                out=xt[:, kt, :ch],
                                in_=xT[:, kt, col : col + ch],
                            )
                        ot = io.tile([128, JT, CHUNK], f16, tag="o", name="o")
                        for j2 in range(JT):
                            for n in range(0, ch, NBLK):
                                nsz = min(NBLK, ch - n)
                                ps = pp.tile(
                                    [128, NBLK], f32, tag="ps", name="ps"
                                )
                                for kt in range(KT):
                                    nc.tensor.matmul(
                                        ps[:, :nsz],
                                        w_sb[:, kt, j2 * 128 : (j2 + 1) * 128],
                                        xt[:, kt, n : n + nsz],
                                        start=(kt == 0),
                                        stop=(kt == KT - 1),
                                    )
                                nc.vector.tensor_scalar_add(
                                    ot[:, j2, n : n + nsz],
                                    ps[:, :nsz],
                                    b_sb[:, j2 : j2 + 1],
                                )
                            nc.scalar.dma_start(
                                out=outT[:, j2, col : col + ch],
                                in_=ot[:, j2, :ch],
                            )
                        col += ch

            if hw_loop > 1:
                with tc.For_i(0, hw_loop):
                    body()
            else:
                body()
    nc.compile()
    return nc


def _get_nc(repeat=1, hw_loop=1):
    key = (repeat, hw_loop)
    if key not in _NC_CACHE:
        _NC_CACHE[key] = _build_nc(repeat, hw_loop)
    return _NC_CACHE[key]


_TBLK = 512  # row-block size for cache-friendly host transposes


def _transpose_cast_into(dst, src):
    """dst[128, rows] (f16) = src[rows, 128].T (f32), blocked."""
    rows = src.shape[0]
    for i in range(0, rows, _TBLK):
        j = min(i + _TBLK, rows)
        dst[:, i:j] = src[i:j].astype(np.float16).T


def _make_core_input(xm, h_sub, c):
    """Per-core activation block [128, KT, R] fp16: xT[p,kt,r]=X[r,kt*128+p]."""
    lo, hi = c * R, (c + 1) * R
    xT_c = np.empty((128, KT, R), np.float16)
    for kt in range(KT):
        src = xm if kt < 2 else h_sub
        c0 = (kt % 2) * 128
        _transpose_cast_into(xT_c[:, kt, :], src[lo:hi, c0 : c0 + 128])
    return xT_c


def _run_device(in_maps):
    _ensure_concourse()
    from concourse.bass_utils import run_bass_kernel_spmd

    nc = _get_nc()
    return run_bass_kernel_spmd(nc, in_maps, list(range(N_CORES))).results


def _copy_rows(dst, src, lo, hi):
    np.copyto(dst[lo:hi], src[lo:hi])


def _untranspose_into(dst, src_t):
    """dst[rows, 256] (f32) = per-j2 transpose of src_t[128, JT, rows] (f16)."""
    rows = dst.shape[0]
    for i in range(0, rows, _TBLK):
        j = min(i + _TBLK, rows)
        for j2 in range(JT):
            dst[i:j, j2 * 128 : (j2 + 1) * 128] = src_t[:, j2, i:j].T
    return dst


def kernel(h_full, h_sub, W1, b1, W2, b2, old_idxs):
    h_full = np.asarray(h_full, dtype=np.float32)
    h_sub = np.asarray(h_sub, dtype=np.float32)
    W1 = np.asarray(W1, dtype=np.float32)
    W2 = np.asarray(W2, dtype=np.float32)
    b1 = np.asarray(b1, dtype=np.float32)
    b2 = np.asarray(b2, dtype=np.float32)
    idx = np.asarray(old_idxs)

    fast = idx.shape == (M,) and bool(
        np.array_equal(idx, np.arange(M, dtype=idx.dtype))
    )
    xm = h_full[:M] if fast else np.ascontiguousarray(h_full[idx])

    # wT[p, kt, j] = Wc.T[kt*128+p, j], Wc = [W1 | W2]
    wcT = np.concatenate([W1.T, W2.T], axis=0).astype(np.float16)
    wT = np.ascontiguousarray(wcT.reshape(KT, 128, DIM).transpose(1, 0, 2))
    bias = np.ascontiguousarray((b1 + b2).astype(np.float32).reshape(JT, 128).T)

    xTs = list(_POOL.map(lambda c: _make_core_input(xm, h_sub, c), range(N_CORES)))
    in_maps = [{"xT": xTs[c], "wT": wT, "bias": bias} for c in range(N_CORES)]

    results = _run_device(in_maps)

    out = np.empty((N, DIM), np.float32)

    if fast:
        jobs = [
            _POOL.submit(
                _untranspose_into, out[c * R : (c + 1) * R], results[c]["outT"]
            )
            for c in range(N_CORES)
        ]
        step = (N - M) // N_CORES
        for c in range(N_CORES):
            lo = M + c * step
            hi = N if c == N_CORES - 1 else M + (c + 1) * step
            jobs.append(_POOL.submit(_copy_rows, out, h_full, lo, hi))
        for j in jobs:
            j.result()
    else:
        merged = np.empty((M, DIM), np.float32)
        list(
            _POOL.map(
                lambda c: _untranspose_into(
                    merged[c * R : (c + 1) * R], results[c]["outT"]
                ),
                range(N_CORES),
            )
        )
        np.copyto(out, h_full)
        out[idx] = merged
    return out
